# revision 4
# baseline (speedup 1.0000x reference)
"""GQA decode attention kernel for Trainium2, tensor-parallel over 8 kv heads.

Contract: kernel(**inputs) takes FULL inputs (numpy), returns FULL output.
Shapes are hardcoded: x[8,16,4096], w_in[6144,4096], w_out[4096,4096],
k_cache[8,4096,8,128], v_cache[8,4096,8,128], offset=4080.

Per-core (core g owns kv head g, q heads 4g..4g+3):
  kv  = x @ w_kv_g.T            -> [128, 256]  (k 128 | v 128)
  q   = x @ w_q_g.T             -> [128, 512]
  rope(q, k); patch new k/v into cache tail (T=4096; new-token k/v
  stay f16: self-attention concentrates softmax mass on them)
  scoresT[t, (r,s)] = kk chunks.T @ qT    (PE, per batch)
  expS = exp(scores)            (ACT; no max-sub: |scores| < ~8)
  denom: log-tree adds on DVE -> [128, 64], then a 64-col matmul
  outT = vv.T @ expS            (PE accumulate); scaled by 1/denom
  partial = attn_out @ w_out[:, 512g:+512].T  -> [128, 4096]
Host sums the 8 partials.

Numerics: f16 activations/weights everywhere (4x less rounding noise
than bf16); k-cache chunks 0-23 (t < 3072) in fp8-e3m4.  v-cache and
the new-token projections stay f16: fp8 there creates outlier errors
under concentrated softmax rows.

Schedule: one sync-queue DMA stream in priority order
  x -> w_kv -> rope tables -> w_q -> per-batch (k8, k16, v halves)
  -> w_out (8 pieces, last).
kv-proj runs first (cheap), so rope-k/patches are ready early; q-proj
chases the w_q pieces and attention starts right as batch 0's kv
lands.  Attention PE per batch (~3.1us) is faster than its kv arrival
(~4.4us), so the stream never throttles on ring reuse.  PV/normalize
lag scores/exp by two batches.  The out-projection chases the w_out
pieces at the stream tail.
"""

import os
import sys

for _p in ("/opt/trn_rl_repo", "/root/.axon_site/_ro/trn_rl_repo"):
    if os.path.isdir(_p) and _p not in sys.path:
        sys.path.insert(0, _p)

import numpy as np
import ml_dtypes

F16 = np.float16
F8 = ml_dtypes.float8_e3m4

B, S, E = 8, 16, 4096
HQ, HKV, HD = 32, 8, 128
R = HQ // HKV          # 4 q heads per kv head
T = 4096               # cache length == offset + S
OFFSET = 4080
NCORES = 8
ROPE_BASE = 10000.0
BS = B * S             # 128 rows
QF = R * HD            # 512 q features per core
KCH = E // 128         # 32 contraction chunks for qkv proj
TCH = T // 128         # 32 T chunks
K8CH = 24              # k-cache chunks stored in fp8 (t < 3072)
K8T = K8CH * 128       # 3072
HT = T // 2            # 2048 cols per v half tile

_CACHED = {}


def _build_program():
    """Build the Bass program once (same program for all cores)."""
    from concourse import bacc, masks, mybir
    from concourse import tile

    f32 = mybir.dt.float32
    f16 = mybir.dt.float16
    f8 = mybir.dt.float8e3
    ActExp = mybir.ActivationFunctionType.Exp

    nc = bacc.Bacc(
        "TRN2",
        target_bir_lowering=False,
        debug=False,
        enable_asserts=False,
        num_devices=NCORES,
    )

    # DRAM I/O (per-core shards, host pre-permuted so every DMA is a plain
    # [128, N] contiguous-per-partition transfer)
    xT_d = nc.dram_tensor("xT", [128, KCH * 128], f16, kind="ExternalInput").ap()
    wq_d = nc.dram_tensor("w_qT", [128, KCH * 512], f16, kind="ExternalInput").ap()
    wkv_d = nc.dram_tensor("w_kvT", [128, KCH * 256], f16, kind="ExternalInput").ap()
    ropec_d = nc.dram_tensor("rope_c", [128, 640], f32, kind="ExternalInput").ap()
    ropes_d = nc.dram_tensor("rope_s", [128, 640], f32, kind="ExternalInput").ap()
    k8_d = nc.dram_tensor("k8", [B, 128, K8T], f8, kind="ExternalInput").ap()
    k16_d = nc.dram_tensor("k16", [B, 128, T - K8T], f16, kind="ExternalInput").ap()
    v_d = nc.dram_tensor("vperm", [B, 128, T], f16, kind="ExternalInput").ap()
    # w_out pre-permuted to [128(d of this core), (n=8, r=4, 512)]
    wout_d = nc.dram_tensor("w_outT", [128, 4 * E], f16, kind="ExternalInput").ap()
    out_d = nc.dram_tensor("out", [BS, E], f16, kind="ExternalOutput").ap()

    with tile.TileContext(nc) as tc:
        from contextlib import ExitStack

        with ExitStack() as ctx:
            const = ctx.enter_context(tc.tile_pool(name="const", bufs=1))
            wqp = ctx.enter_context(tc.tile_pool(name="wqp", bufs=1))
            wkvp = ctx.enter_context(tc.tile_pool(name="wkvp", bufs=1))
            woutp = ctx.enter_context(tc.tile_pool(name="woutp", bufs=1))
            work = ctx.enter_context(tc.tile_pool(name="work", bufs=1))
            k8pool = ctx.enter_context(tc.tile_pool(name="k8pool", bufs=5))
            k16pool = ctx.enter_context(tc.tile_pool(name="k16pool", bufs=5))
            vpool = ctx.enter_context(tc.tile_pool(name="vpool", bufs=4))
            epool = ctx.enter_context(tc.tile_pool(name="epool", bufs=6))
            dtree = ctx.enter_context(tc.tile_pool(name="dtree", bufs=1))
            esump = ctx.enter_context(tc.tile_pool(name="esump", bufs=4))
            bcp = ctx.enter_context(tc.tile_pool(name="bcp", bufs=2))
            opool = ctx.enter_context(tc.tile_pool(name="opool", bufs=3))
            ps_big = ctx.enter_context(
                tc.tile_pool(name="ps_big", bufs=4, space="PSUM")
            )
            ps_out = ctx.enter_context(
                tc.tile_pool(name="ps_out", bufs=3, space="PSUM")
            )
            ps_sm = ctx.enter_context(tc.tile_pool(name="ps_sm", bufs=1, space="PSUM"))

            # ---- all input DMAs, one hardware queue (sync), priority order.
            x_p = [
                const.tile([128, 8 * 128], f16, tag=f"x{j}", name=f"x{j}")
                for j in range(4)
            ]
            wkv_p = [
                wkvp.tile([128, 8 * 256], f16, tag=f"wkv{j}", name=f"wkv{j}")
                for j in range(4)
            ]
            wq_p = [
                wqp.tile([128, 8 * 512], f16, tag=f"wq{j}", name=f"wq{j}")
                for j in range(4)
            ]
            ropeC = const.tile([128, 640], f32, tag="ropeC")
            ropeS = const.tile([128, 640], f32, tag="ropeS")
            # x/w_kv/w_q + rope tables go on the scalar HW queue so the kv
            # cache stream owns the sync queue from t=0 (two queues double
            # the in-flight descriptor window); these issues are all early,
            # so they never block the exp ACTs behind them
            for j in range(4):
                nc.scalar.dma_start(x_p[j][:], xT_d[:, j * 1024 : (j + 1) * 1024])
                nc.scalar.dma_start(wkv_p[j][:], wkv_d[:, j * 2048 : (j + 1) * 2048])
            nc.scalar.dma_start(ropeC[:], ropec_d[:])
            nc.scalar.dma_start(ropeS[:], ropes_d[:])
            for j in range(4):
                nc.scalar.dma_start(wq_p[j][:], wq_d[:, j * 4096 : (j + 1) * 4096])

            # kv cache per batch; w_out streams last (the out-projection
            # chases its pieces at the very end)
            k8_t = [None] * B
            k16_t = [None] * B
            vv_t = [None] * B
            for b in range(B):
                k8_t[b] = k8pool.tile([128, K8T], f8, tag="k8", name=f"k8_{b}")
                nc.sync.dma_start(k8_t[b][:], k8_d[b])
                k16_t[b] = k16pool.tile(
                    [128, T - K8T], f16, tag="k16", name=f"k16_{b}"
                )
                nc.sync.dma_start(k16_t[b][:], k16_d[b])
                vv_t[b] = vpool.tile([128, T], f16, tag="vv", name=f"vv{b}")
                nc.sync.dma_start(vv_t[b][:], v_d[b])
            w_outT = [None] * 8
            for n in range(8):
                w_outT[n] = woutp.tile([128, 2048], f16, tag=f"wo{n}", name=f"wo{n}")
                nc.sync.dma_start(w_outT[n][:], wout_d[:, n * 2048 : (n + 1) * 2048])

            # ---- constants (vector/gpsimd engines; do not block the DMA queue)
            ident = const.tile([128, 128], f32, tag="ident")
            masks.make_identity(nc, ident[:])
            ones_col = const.tile([128, 1], f16, tag="ones_col")
            nc.vector.memset(ones_col[:], 1.0)
            ones_row = const.tile([1, 128], f32, tag="ones_row")
            nc.vector.memset(ones_row[:], 1.0)

            # ---- phase 1a: k|v projection in psum (cheap, runs first so the
            # rope-k / cache patches are ready before attention starts)
            kv_ps = ps_sm.tile([128, 256], f32, tag="sm", name="kv_ps")
            for k in range(KCH):
                j, kk_ = k // 8, k % 8
                nc.tensor.matmul(
                    kv_ps[:],
                    x_p[j][:, kk_ * 128 : (kk_ + 1) * 128],
                    wkv_p[j][:, kk_ * 256 : (kk_ + 1) * 256],
                    start=(k == 0),
                    stop=(k == KCH - 1),
                )

            # rope-k: out = k*C + rot(k)*S, tables carry the 1/WKV_SCALE
            rot = work.tile([128, 640], f32, tag="rot")
            nc.scalar.copy(rot[:, 576:640], kv_ps[:, 0:64])
            nc.scalar.copy(rot[:, 512:576], kv_ps[:, 64:128])
            ropeCs = work.tile([128, 640], f32, tag="ropeCs")
            nc.gpsimd.tensor_copy(ropeCs[:], ropeC[:])
            ropeSs = work.tile([128, 640], f32, tag="ropeSs")
            nc.gpsimd.tensor_copy(ropeSs[:], ropeS[:])
            roped = work.tile([128, 640], f32, tag="roped")
            nc.vector.tensor_mul(roped[:, 512:640], kv_ps[:, 0:128], ropeCs[:, 512:640])
            t2 = work.tile([128, 640], f32, tag="t2")
            nc.vector.tensor_mul(t2[:, 512:640], rot[:, 512:640], ropeSs[:, 512:640])
            nc.vector.tensor_add(roped[:, 512:640], roped[:, 512:640], t2[:, 512:640])

            # v_sb: new-token v rows [128(bs), 128(d)], unscaled (1/WKV_SCALE)
            v_sb = work.tile([128, 128], f16, tag="v_sb")
            nc.scalar.copy(v_sb[:], kv_ps[:, 128:256])

            # ---- phase 1b: q projection, piece-paced against the w_q stream
            q_ps = ps_big.tile([128, 512], f32, tag="sc", name="q_ps")
            for k in range(KCH):
                j, kk_ = k // 8, k % 8
                nc.tensor.matmul(
                    q_ps[:],
                    x_p[j][:, kk_ * 128 : (kk_ + 1) * 128],
                    wq_p[j][:, kk_ * 512 : (kk_ + 1) * 512],
                    start=(k == 0),
                    stop=(k == KCH - 1),
                )

            # transpose new k -> kT_sb [128(d), (b s)] f16
            kT_sb = work.tile([128, 128], f16, tag="kT_sb")
            tpk = ps_out.tile([128, 128], f32, tag="po")
            nc.tensor.transpose(tpk[:], roped[:, 512:640], ident[:])
            nc.vector.tensor_copy(kT_sb[:], tpk[:])

            # rope-q (tables carry the attention scale): rotate-half copies on
            # scalar, multiplies on vector
            rot4 = rot[:].rearrange("p (blk h j) -> p blk h j", blk=5, h=2)
            ps4 = q_ps[:].rearrange("p (blk h j) -> p blk h j", blk=4, h=2)
            nc.scalar.copy(rot4[:, 0:4, 0, :], ps4[:, :, 1, :])
            nc.scalar.copy(rot4[:, 0:4, 1, :], ps4[:, :, 0, :])
            nc.vector.tensor_mul(roped[:, 0:512], q_ps[:], ropeCs[:, 0:512])
            nc.vector.tensor_mul(t2[:, 0:512], rot[:, 0:512], ropeSs[:, 0:512])
            nc.vector.tensor_add(roped[:, 0:512], roped[:, 0:512], t2[:, 0:512])

            # transpose q heads -> q_allT [128(d), (b r s)] f16
            q_allT = work.tile([128, B * R * S], f16, tag="q_allT")
            qv = q_allT[:].rearrange("p (b r s) -> p b r s", b=B, r=R)
            for r in range(R):
                tp = ps_out.tile([128, 128], f32, tag="po")
                nc.tensor.transpose(tp[:], roped[:, r * 128 : (r + 1) * 128], ident[:])
                nc.vector.tensor_copy(
                    qv[:, :, r, :], tp[:].rearrange("p (b s) -> p b s", b=B)
                )

            # attention output, [128(d), (r, b, s)] f16
            attn_T = work.tile([128, R * BS], f16, tag="attn", name="attn")

            expS = [None] * B
            esum = [None] * B

            def k_chunk(b, c):
                if c < K8CH:
                    return k8_t[b][:, c * 128 : (c + 1) * 128]
                cc = c - K8CH
                return k16_t[b][:, cc * 128 : (cc + 1) * 128]

            def emit_out_pair(m):
                # two 512-col slices -> one [128, 1024] tile -> one DMA
                out_sb = opool.tile([128, 1024], f16, tag="out_sb", name="out_sb")
                for half in range(2):
                    n = 2 * m + half
                    part_ps = ps_out.tile([128, 512], f32, tag="po", name="part_ps")
                    for r in range(R):
                        nc.tensor.matmul(
                            part_ps[:],
                            attn_T[:, r * BS : (r + 1) * BS],
                            w_outT[n][:, r * 512 : (r + 1) * 512],
                            start=(r == 0),
                            stop=(r == R - 1),
                        )
                    o = out_sb[:, half * 512 : (half + 1) * 512]
                    # split each psum->f16 cast across vector+scalar
                    nc.vector.tensor_copy(o[:, 0:320], part_ps[:, 0:320])
                    nc.scalar.copy(o[:, 320:512], part_ps[:, 320:512])
                nc.gpsimd.dma_start(
                    out_d[:, m * 1024 : (m + 1) * 1024],
                    out_sb[:],
                )

            # ---- phase 2: attention, PV/normalize lagging scores/exp by two
            # batches so the PE stream never waits on the activation engine
            for b in range(B + 2):
                if b < B:
                    # patch stale tail keys/values with roped new ones (the v
                    # patch is an SBUF->SBUF DMA: compute engines need
                    # quadrant-aligned partition starts, DMA does not)
                    nc.vector.tensor_copy(
                        k16_t[b][:, OFFSET - K8T : T - K8T],
                        kT_sb[:, b * S : (b + 1) * S],
                    )
                    nc.gpsimd.dma_start(
                        vv_t[b][112:128, 31 * 128 : 32 * 128],
                        v_sb[b * S : (b + 1) * S, :],
                    )
                    expS[b] = epool.tile([128, TCH * 64], f16, tag="expS", name=f"e{b}")
                    # scores in 4 one-bank psum quarters (ring of 4): the PE
                    # can run up to 3 quarters ahead of the exp ACTs
                    for qtr in range(4):
                        sc = ps_big.tile([128, 512], f32, tag="sc", name=f"sc{b}_{qtr}")
                        for tt in range(8):
                            c = qtr * 8 + tt
                            nc.tensor.matmul(
                                sc[:, tt * 64 : (tt + 1) * 64],
                                k_chunk(b, c),
                                q_allT[:, b * 64 : (b + 1) * 64],
                                start=True,
                                stop=True,
                            )
                        nc.scalar.activation(
                            expS[b][:, qtr * 512 : (qtr + 1) * 512],
                            sc[:],
                            ActExp,
                        )
                    # denominator log-tree on the vector engine: [128, 2048]
                    # -> [128, 64] partial sums (over the chunk index), so the
                    # PE only pays a single 64-col matmul per batch
                    sA = dtree.tile([128, 1024], f16, tag="sA", name=f"sA{b}")
                    sB = dtree.tile([128, 512], f16, tag="sB", name=f"sB{b}")
                    nc.vector.tensor_add(
                        sA[:], expS[b][:, 0:1024], expS[b][:, 1024:2048]
                    )
                    nc.vector.tensor_add(sB[:], sA[:, 0:512], sA[:, 512:1024])
                    nc.vector.tensor_add(sA[:, 0:256], sB[:, 0:256], sB[:, 256:512])
                    nc.vector.tensor_add(sB[:, 0:128], sA[:, 0:128], sA[:, 128:256])
                    esum[b] = esump.tile([128, 64], f16, tag="esum", name=f"es{b}")
                    nc.vector.tensor_add(esum[b][:], sB[:, 0:64], sB[:, 64:128])
                if b > 1:
                    pb = b - 2
                    # denom first: its reciprocal computes on vector while the
                    # PE streams PV
                    denom_ps = ps_sm.tile([1, 64], f32, tag="sm", name=f"d{pb}")
                    nc.tensor.matmul(
                        denom_ps[:], ones_col[:], esum[pb][:], start=True, stop=True
                    )
                    recip = bcp.tile([1, 64], f32, tag="recip", name=f"r{pb}")
                    nc.vector.reciprocal(recip[:], denom_ps[:])
                    outT_ps = ps_out.tile([128, 64], f32, tag="po", name=f"o{pb}")
                    for c in range(TCH):
                        nc.tensor.matmul(
                            outT_ps[:],
                            vv_t[pb][:, c * 128 : (c + 1) * 128],
                            expS[pb][:, c * 64 : (c + 1) * 64],
                            start=(c == 0),
                            stop=(c == TCH - 1),
                        )
                    bc_ps = ps_sm.tile([128, 64], f32, tag="sm", name=f"bc{pb}")
                    nc.tensor.matmul(
                        bc_ps[:], ones_row[:], recip[:], start=True, stop=True
                    )
                    bc_sb = bcp.tile([128, 64], f32, tag="bc_sb", name=f"bc_sb{pb}")
                    nc.vector.tensor_copy(bc_sb[:], bc_ps[:])
                    # single strided scale: attn[(r, pb, s)] = outT[(r, s)] * bc
                    attn_view = attn_T[:].rearrange("p (r b s) -> p r b s", r=R, b=B)
                    nc.vector.tensor_mul(
                        attn_view[:, :, pb, :],
                        outT_ps[:].rearrange("p (r s) -> p r s", r=R),
                        bc_sb[:].rearrange("p (r s) -> p r s", r=R),
                    )
                    if pb == B - 1:
                        for m in range(4):
                            emit_out_pair(m)

    nc.compile()
    return nc


def _host_shards(x, w_in, w_out, k_cache, v_cache):
    """Per-core input dicts, pre-permuted for contiguous [128, N] DMAs."""
    x2 = np.ascontiguousarray(x.reshape(BS, E))
    xT_perm = (
        x2.T.reshape(KCH, 128, 128).transpose(1, 0, 2).reshape(128, KCH * 128)
    ).astype(F16)

    # rope tables (identical on all cores); q blocks carry the attention
    # scale, the k block carries the w_kv fp8 unscale
    inv_freq = 1.0 / (ROPE_BASE ** (np.arange(0, HD, 2, dtype=np.float64) / HD))
    pos = (OFFSET + np.arange(S)).astype(np.float64)
    ang = pos[:, None] * inv_freq[None, :]
    cos16 = np.cos(ang).astype(np.float32)
    sin16 = np.sin(ang).astype(np.float32)
    scale = np.float32(1.0 / np.sqrt(HD))
    C = np.zeros((128, 640), np.float32)
    Sn = np.zeros((128, 640), np.float32)
    srow = np.arange(128) % S                        # partition p=(b,s) -> s
    for blk in range(5):
        blk_scale = scale if blk < 4 else np.float32(1.0)
        C[:, blk * 128 : blk * 128 + 64] = cos16[srow] * blk_scale
        C[:, blk * 128 + 64 : blk * 128 + 128] = cos16[srow] * blk_scale
        Sn[:, blk * 128 : blk * 128 + 64] = -sin16[srow] * blk_scale
        Sn[:, blk * 128 + 64 : blk * 128 + 128] = sin16[srow] * blk_scale

    shards = []
    for g in range(NCORES):
        wq = w_in[QF * g : QF * (g + 1)]             # [512, 4096]
        wqT = (
            wq.T.reshape(KCH, 128, 512).transpose(1, 0, 2).reshape(128, KCH * 512)
        ).astype(F16)
        wkv = np.concatenate(
            [
                w_in[E + HD * g : E + HD * (g + 1)],
                w_in[E + HKV * HD + HD * g : E + HKV * HD + HD * (g + 1)],
            ],
            axis=0,
        )                                            # [256, 4096]
        wkvT = (
            wkv.T.reshape(KCH, 128, 256).transpose(1, 0, 2).reshape(128, KCH * 256)
        ).astype(F16)
        # [128(d), (r, e)] -> [128(d), (n, r, 512)] so out-proj slice n is
        # contiguous
        w_outT_perm = (
            w_out[:, QF * g : QF * (g + 1)]
            .T.reshape(4, 128, E)
            .transpose(1, 0, 2)
            .reshape(128, 4, 8, 512)
            .transpose(0, 2, 1, 3)
            .reshape(128, 4 * E)
        ).astype(F16)
        kT = np.ascontiguousarray(
            k_cache[:, :, g, :].transpose(0, 2, 1)
        )  # [B, 128(d), T]
        k8 = kT[:, :, :K8T].astype(F8)
        k16 = kT[:, :, K8T:].astype(F16)
        vperm = np.ascontiguousarray(
            v_cache[:, :, g, :]
            .reshape(B, TCH, 128, HD)
            .transpose(0, 2, 1, 3)
            .reshape(B, 128, T)
        ).astype(F16)  # [B, 128(t_in), (chunk d)]
        shards.append(
            {
                "xT": xT_perm,
                "w_qT": wqT,
                "w_kvT": wkvT,
                "w_outT": w_outT_perm,
                "rope_c": C,
                "rope_s": Sn,
                "k8": np.ascontiguousarray(k8),
                "k16": np.ascontiguousarray(k16),
                "vperm": vperm,
            }
        )
    return shards


def _get_nc():
    if "nc" not in _CACHED:
        _CACHED["nc"] = _build_program()
    return _CACHED["nc"]


def run_on_hw(in_maps, trace=False, **kw):
    from concourse import bass_utils

    nc = _get_nc()
    return bass_utils.run_bass_kernel_spmd(
        nc, in_maps, core_ids=list(range(NCORES)), trace=trace, **kw
    )


def kernel(x, w_in, w_out, k_cache, v_cache, offset):
    assert int(offset) == OFFSET and x.shape == (B, S, E)
    shards = _host_shards(
        np.asarray(x, np.float32),
        np.asarray(w_in, np.float32),
        np.asarray(w_out, np.float32),
        np.asarray(k_cache, np.float32),
        np.asarray(v_cache, np.float32),
    )
    res = run_on_hw(shards)
    out = np.zeros((BS, E), np.float64)
    for g in range(NCORES):
        out += np.asarray(res.results[g]["out"], np.float64)
    return out.astype(np.float32).reshape(B, S, E)


# revision 5
# speedup vs baseline: 1.1324x; 1.1324x over previous
"""GQA decode attention kernel for Trainium2, tensor-parallel over 8 kv heads.

Contract: kernel(**inputs) takes FULL inputs (numpy), returns FULL output.
Shapes are hardcoded: x[8,16,4096], w_in[6144,4096], w_out[4096,4096],
k_cache[8,4096,8,128], v_cache[8,4096,8,128], offset=4080.

Per-core (core g owns kv head g, q heads 4g..4g+3):
  kv  = x @ w_kv_g.T            -> [128, 256]  (k 128 | v 128)
  q   = x @ w_q_g.T             -> [128, 512]
  rope(q, k); patch new k/v into cache tail (T=4096; new-token k/v
  stay f16: self-attention concentrates softmax mass on them)
  scoresT[t, (r,s)] = kk chunks.T @ qT    (PE, per batch)
  expS = exp(scores)            (ACT; no max-sub: |scores| < ~8)
  denom: log-tree adds on DVE -> [128, 64], then a 64-col matmul
  outT = vv.T @ expS            (PE accumulate); scaled by 1/denom
  partial = attn_out @ w_out[:, 512g:+512].T  -> [128, 4096]
Host sums the 8 partials.

Numerics: f16 activations/weights everywhere (4x less rounding noise
than bf16); k-cache chunks 0-23 (t < 3072) in fp8-e3m4.  v-cache and
the new-token projections stay f16: fp8 there creates outlier errors
under concentrated softmax rows.

Schedule: one sync-queue DMA stream in priority order
  x -> w_kv -> rope tables -> w_q -> per-batch (k8, k16, v halves)
  -> w_out (8 pieces, last).
kv-proj runs first (cheap), so rope-k/patches are ready early; q-proj
chases the w_q pieces and attention starts right as batch 0's kv
lands.  Attention PE per batch (~3.1us) is faster than its kv arrival
(~4.4us), so the stream never throttles on ring reuse.  PV/normalize
lag scores/exp by two batches.  The out-projection chases the w_out
pieces at the stream tail.
"""

import os
import sys

for _p in ("/opt/trn_rl_repo", "/root/.axon_site/_ro/trn_rl_repo"):
    if os.path.isdir(_p) and _p not in sys.path:
        sys.path.insert(0, _p)

import numpy as np
import ml_dtypes

F16 = np.float16
F8 = ml_dtypes.float8_e3m4

B, S, E = 8, 16, 4096
HQ, HKV, HD = 32, 8, 128
R = HQ // HKV          # 4 q heads per kv head
T = 4096               # cache length == offset + S
OFFSET = 4080
NCORES = 8
ROPE_BASE = 10000.0
BS = B * S             # 128 rows
QF = R * HD            # 512 q features per core
KCH = E // 128         # 32 contraction chunks for qkv proj
TCH = T // 128         # 32 T chunks
K8CH = 24              # k-cache chunks stored in fp8 (t < 3072)
K8T = K8CH * 128       # 3072
HT = T // 2            # 2048 cols per v half tile

_CACHED = {}


def _build_program():
    """Build the Bass program once (same program for all cores)."""
    from concourse import bacc, masks, mybir
    from concourse import tile

    f32 = mybir.dt.float32
    f16 = mybir.dt.float16
    f8 = mybir.dt.float8e3
    ActExp = mybir.ActivationFunctionType.Exp

    nc = bacc.Bacc(
        "TRN2",
        target_bir_lowering=False,
        debug=False,
        enable_asserts=False,
        num_devices=NCORES,
    )

    # DRAM I/O (per-core shards, host pre-permuted so every DMA is a plain
    # [128, N] contiguous-per-partition transfer)
    xT_d = nc.dram_tensor("xT", [128, KCH * 128], f16, kind="ExternalInput").ap()
    wq_d = nc.dram_tensor("w_qT", [128, KCH * 512], f16, kind="ExternalInput").ap()
    wkv_d = nc.dram_tensor("w_kvT", [128, KCH * 256], f16, kind="ExternalInput").ap()
    ropec_d = nc.dram_tensor("rope_c", [128, 640], f32, kind="ExternalInput").ap()
    ropes_d = nc.dram_tensor("rope_s", [128, 640], f32, kind="ExternalInput").ap()
    k8_d = nc.dram_tensor("k8", [B, 128, K8T], f8, kind="ExternalInput").ap()
    k16_d = nc.dram_tensor("k16", [B, 128, T - K8T], f16, kind="ExternalInput").ap()
    v_d = nc.dram_tensor("vperm", [B, 128, T], f16, kind="ExternalInput").ap()
    # w_out pre-permuted to [128(d of this core), (n=8, r=4, 512)]
    wout_d = nc.dram_tensor("w_outT", [128, 4 * E], f16, kind="ExternalInput").ap()
    out_d = nc.dram_tensor("out", [BS, E], f16, kind="ExternalOutput").ap()

    with tile.TileContext(nc) as tc:
        from contextlib import ExitStack

        with ExitStack() as ctx:
            const = ctx.enter_context(tc.tile_pool(name="const", bufs=1))
            wqp = ctx.enter_context(tc.tile_pool(name="wqp", bufs=1))
            wkvp = ctx.enter_context(tc.tile_pool(name="wkvp", bufs=1))
            woutp = ctx.enter_context(tc.tile_pool(name="woutp", bufs=1))
            work = ctx.enter_context(tc.tile_pool(name="work", bufs=1))
            k8pool = ctx.enter_context(tc.tile_pool(name="k8pool", bufs=5))
            k16pool = ctx.enter_context(tc.tile_pool(name="k16pool", bufs=5))
            vpool = ctx.enter_context(tc.tile_pool(name="vpool", bufs=4))
            epool = ctx.enter_context(tc.tile_pool(name="epool", bufs=6))
            dtree = ctx.enter_context(tc.tile_pool(name="dtree", bufs=1))
            esump = ctx.enter_context(tc.tile_pool(name="esump", bufs=4))
            bcp = ctx.enter_context(tc.tile_pool(name="bcp", bufs=2))
            opool = ctx.enter_context(tc.tile_pool(name="opool", bufs=3))
            ps_big = ctx.enter_context(
                tc.tile_pool(name="ps_big", bufs=4, space="PSUM")
            )
            ps_out = ctx.enter_context(
                tc.tile_pool(name="ps_out", bufs=3, space="PSUM")
            )
            ps_sm = ctx.enter_context(tc.tile_pool(name="ps_sm", bufs=1, space="PSUM"))

            # ---- all input DMAs, one hardware queue (sync), priority order.
            x_p = [
                const.tile([128, 8 * 128], f16, tag=f"x{j}", name=f"x{j}")
                for j in range(4)
            ]
            wkv_p = [
                wkvp.tile([128, 8 * 256], f16, tag=f"wkv{j}", name=f"wkv{j}")
                for j in range(4)
            ]
            wq_p = [
                wqp.tile([128, 8 * 512], f16, tag=f"wq{j}", name=f"wq{j}")
                for j in range(4)
            ]
            ropeC = const.tile([128, 640], f32, tag="ropeC")
            ropeS = const.tile([128, 640], f32, tag="ropeS")
            # x + w_kv interleaved: the kv projection starts immediately.
            # Everything on ONE queue: weights must finish at full bandwidth
            # before kv (attention start is gated on w_q), and a second queue
            # has no priority control.
            for j in range(4):
                nc.sync.dma_start(x_p[j][:], xT_d[:, j * 1024 : (j + 1) * 1024])
                nc.sync.dma_start(wkv_p[j][:], wkv_d[:, j * 2048 : (j + 1) * 2048])
            nc.sync.dma_start(ropeC[:], ropec_d[:])
            nc.sync.dma_start(ropeS[:], ropes_d[:])
            for j in range(4):
                nc.sync.dma_start(wq_p[j][:], wq_d[:, j * 4096 : (j + 1) * 4096])

            # kv cache per batch; w_out streams last (the out-projection
            # chases its pieces at the very end)
            k8_t = [None] * B
            k16_t = [None] * B
            vv_t = [None] * B
            for b in range(B):
                k8_t[b] = k8pool.tile([128, K8T], f8, tag="k8", name=f"k8_{b}")
                nc.sync.dma_start(k8_t[b][:], k8_d[b])
                k16_t[b] = k16pool.tile(
                    [128, T - K8T], f16, tag="k16", name=f"k16_{b}"
                )
                nc.sync.dma_start(k16_t[b][:], k16_d[b])
                vv_t[b] = vpool.tile([128, T], f16, tag="vv", name=f"vv{b}")
                nc.sync.dma_start(vv_t[b][:], v_d[b])
            w_outT = [None] * 8
            for n in range(8):
                w_outT[n] = woutp.tile([128, 2048], f16, tag=f"wo{n}", name=f"wo{n}")
                nc.sync.dma_start(w_outT[n][:], wout_d[:, n * 2048 : (n + 1) * 2048])

            # ---- constants (vector/gpsimd engines; do not block the DMA queue)
            ident = const.tile([128, 128], f32, tag="ident")
            masks.make_identity(nc, ident[:])
            ones_col = const.tile([128, 1], f16, tag="ones_col")
            nc.vector.memset(ones_col[:], 1.0)
            ones_row = const.tile([1, 128], f32, tag="ones_row")
            nc.vector.memset(ones_row[:], 1.0)

            # ---- phase 1a: k|v projection in psum (cheap, runs first so the
            # rope-k / cache patches are ready before attention starts)
            kv_ps = ps_sm.tile([128, 256], f32, tag="sm", name="kv_ps")
            for k in range(KCH):
                j, kk_ = k // 8, k % 8
                nc.tensor.matmul(
                    kv_ps[:],
                    x_p[j][:, kk_ * 128 : (kk_ + 1) * 128],
                    wkv_p[j][:, kk_ * 256 : (kk_ + 1) * 256],
                    start=(k == 0),
                    stop=(k == KCH - 1),
                )

            # rope-k: out = k*C + rot(k)*S, tables carry the 1/WKV_SCALE
            rot = work.tile([128, 640], f32, tag="rot")
            nc.scalar.copy(rot[:, 576:640], kv_ps[:, 0:64])
            nc.scalar.copy(rot[:, 512:576], kv_ps[:, 64:128])
            ropeCs = work.tile([128, 640], f32, tag="ropeCs")
            nc.gpsimd.tensor_copy(ropeCs[:], ropeC[:])
            ropeSs = work.tile([128, 640], f32, tag="ropeSs")
            nc.gpsimd.tensor_copy(ropeSs[:], ropeS[:])
            roped = work.tile([128, 640], f32, tag="roped")
            nc.vector.tensor_mul(roped[:, 512:640], kv_ps[:, 0:128], ropeCs[:, 512:640])
            t2 = work.tile([128, 640], f32, tag="t2")
            nc.vector.tensor_mul(t2[:, 512:640], rot[:, 512:640], ropeSs[:, 512:640])
            nc.vector.tensor_add(roped[:, 512:640], roped[:, 512:640], t2[:, 512:640])

            # v_sb: new-token v rows [128(bs), 128(d)], unscaled (1/WKV_SCALE)
            v_sb = work.tile([128, 128], f16, tag="v_sb")
            nc.scalar.copy(v_sb[:], kv_ps[:, 128:256])

            # ---- phase 1b: q projection, piece-paced against the w_q stream
            q_ps = ps_big.tile([128, 512], f32, tag="sc", name="q_ps")
            for k in range(KCH):
                j, kk_ = k // 8, k % 8
                nc.tensor.matmul(
                    q_ps[:],
                    x_p[j][:, kk_ * 128 : (kk_ + 1) * 128],
                    wq_p[j][:, kk_ * 512 : (kk_ + 1) * 512],
                    start=(k == 0),
                    stop=(k == KCH - 1),
                )

            # transpose new k -> kT_sb [128(d), (b s)] f16
            kT_sb = work.tile([128, 128], f16, tag="kT_sb")
            tpk = ps_out.tile([128, 128], f32, tag="po")
            nc.tensor.transpose(tpk[:], roped[:, 512:640], ident[:])
            nc.vector.tensor_copy(kT_sb[:], tpk[:])

            # rope-q (tables carry the attention scale): rotate-half copies on
            # scalar, multiplies on vector
            rot4 = rot[:].rearrange("p (blk h j) -> p blk h j", blk=5, h=2)
            ps4 = q_ps[:].rearrange("p (blk h j) -> p blk h j", blk=4, h=2)
            nc.scalar.copy(rot4[:, 0:4, 0, :], ps4[:, :, 1, :])
            nc.scalar.copy(rot4[:, 0:4, 1, :], ps4[:, :, 0, :])
            nc.vector.tensor_mul(roped[:, 0:512], q_ps[:], ropeCs[:, 0:512])
            nc.vector.tensor_mul(t2[:, 0:512], rot[:, 0:512], ropeSs[:, 0:512])
            nc.vector.tensor_add(roped[:, 0:512], roped[:, 0:512], t2[:, 0:512])

            # transpose q heads -> q_allT [128(d), (b r s)] f16
            q_allT = work.tile([128, B * R * S], f16, tag="q_allT")
            qv = q_allT[:].rearrange("p (b r s) -> p b r s", b=B, r=R)
            for r in range(R):
                tp = ps_out.tile([128, 128], f32, tag="po")
                nc.tensor.transpose(tp[:], roped[:, r * 128 : (r + 1) * 128], ident[:])
                nc.vector.tensor_copy(
                    qv[:, :, r, :], tp[:].rearrange("p (b s) -> p b s", b=B)
                )

            # attention output, [128(d), (r, b, s)] f16
            attn_T = work.tile([128, R * BS], f16, tag="attn", name="attn")

            expS = [None] * B
            esum = [None] * B

            def k_chunk(b, c):
                if c < K8CH:
                    return k8_t[b][:, c * 128 : (c + 1) * 128]
                cc = c - K8CH
                return k16_t[b][:, cc * 128 : (cc + 1) * 128]

            def emit_out_pair(m):
                # two 512-col slices -> one [128, 1024] tile -> one DMA
                out_sb = opool.tile([128, 1024], f16, tag="out_sb", name="out_sb")
                for half in range(2):
                    n = 2 * m + half
                    part_ps = ps_out.tile([128, 512], f32, tag="po", name="part_ps")
                    for r in range(R):
                        nc.tensor.matmul(
                            part_ps[:],
                            attn_T[:, r * BS : (r + 1) * BS],
                            w_outT[n][:, r * 512 : (r + 1) * 512],
                            start=(r == 0),
                            stop=(r == R - 1),
                        )
                    o = out_sb[:, half * 512 : (half + 1) * 512]
                    # split each psum->f16 cast across vector+scalar
                    nc.vector.tensor_copy(o[:, 0:320], part_ps[:, 0:320])
                    nc.scalar.copy(o[:, 320:512], part_ps[:, 320:512])
                nc.gpsimd.dma_start(
                    out_d[:, m * 1024 : (m + 1) * 1024],
                    out_sb[:],
                )

            # ---- phase 2: attention, PV/normalize lagging scores/exp by two
            # batches so the PE stream never waits on the activation engine
            for b in range(B + 2):
                if b > 1:
                    pb = b - 2
                    # lagged stage first: when scores(b) would stall on the
                    # k-cache DMA, the in-order PE still has PV(b-2) to run
                    denom_ps = ps_sm.tile([1, 64], f32, tag="sm", name=f"d{pb}")
                    nc.tensor.matmul(
                        denom_ps[:], ones_col[:], esum[pb][:], start=True, stop=True
                    )
                    recip = bcp.tile([1, 64], f32, tag="recip", name=f"r{pb}")
                    nc.vector.reciprocal(recip[:], denom_ps[:])
                    outT_ps = ps_out.tile([128, 64], f32, tag="po", name=f"o{pb}")
                    for c in range(TCH):
                        nc.tensor.matmul(
                            outT_ps[:],
                            vv_t[pb][:, c * 128 : (c + 1) * 128],
                            expS[pb][:, c * 64 : (c + 1) * 64],
                            start=(c == 0),
                            stop=(c == TCH - 1),
                        )
                    bc_ps = ps_sm.tile([128, 64], f32, tag="sm", name=f"bc{pb}")
                    nc.tensor.matmul(
                        bc_ps[:], ones_row[:], recip[:], start=True, stop=True
                    )
                    bc_sb = bcp.tile([128, 64], f32, tag="bc_sb", name=f"bc_sb{pb}")
                    nc.vector.tensor_copy(bc_sb[:], bc_ps[:])
                    # single strided scale: attn[(r, pb, s)] = outT[(r, s)] * bc
                    attn_view = attn_T[:].rearrange("p (r b s) -> p r b s", r=R, b=B)
                    nc.vector.tensor_mul(
                        attn_view[:, :, pb, :],
                        outT_ps[:].rearrange("p (r s) -> p r s", r=R),
                        bc_sb[:].rearrange("p (r s) -> p r s", r=R),
                    )
                if b < B:
                    # patch stale tail keys/values with roped new ones (the v
                    # patch is an SBUF->SBUF DMA: compute engines need
                    # quadrant-aligned partition starts, DMA does not)
                    nc.vector.tensor_copy(
                        k16_t[b][:, OFFSET - K8T : T - K8T],
                        kT_sb[:, b * S : (b + 1) * S],
                    )
                    nc.gpsimd.dma_start(
                        vv_t[b][112:128, 31 * 128 : 32 * 128],
                        v_sb[b * S : (b + 1) * S, :],
                    )
                    expS[b] = epool.tile([128, TCH * 64], f16, tag="expS", name=f"e{b}")
                    # scores in 4 one-bank psum quarters (ring of 4): the PE
                    # can run up to 3 quarters ahead of the exp ACTs
                    for qtr in range(4):
                        sc = ps_big.tile([128, 512], f32, tag="sc", name=f"sc{b}_{qtr}")
                        for tt in range(8):
                            c = qtr * 8 + tt
                            nc.tensor.matmul(
                                sc[:, tt * 64 : (tt + 1) * 64],
                                k_chunk(b, c),
                                q_allT[:, b * 64 : (b + 1) * 64],
                                start=True,
                                stop=True,
                            )
                        nc.scalar.activation(
                            expS[b][:, qtr * 512 : (qtr + 1) * 512],
                            sc[:],
                            ActExp,
                        )
                    # denominator log-tree on the vector engine: [128, 2048]
                    # -> [128, 64] partial sums (over the chunk index), so the
                    # PE only pays a single 64-col matmul per batch
                    sA = dtree.tile([128, 1024], f16, tag="sA", name=f"sA{b}")
                    sB = dtree.tile([128, 512], f16, tag="sB", name=f"sB{b}")
                    nc.vector.tensor_add(
                        sA[:], expS[b][:, 0:1024], expS[b][:, 1024:2048]
                    )
                    nc.vector.tensor_add(sB[:], sA[:, 0:512], sA[:, 512:1024])
                    nc.vector.tensor_add(sA[:, 0:256], sB[:, 0:256], sB[:, 256:512])
                    nc.vector.tensor_add(sB[:, 0:128], sA[:, 0:128], sA[:, 128:256])
                    esum[b] = esump.tile([128, 64], f16, tag="esum", name=f"es{b}")
                    nc.vector.tensor_add(esum[b][:], sB[:, 0:64], sB[:, 64:128])
                if b > 1:
                    pb = b - 2
                    if pb == B - 1:
                        for m in range(4):
                            emit_out_pair(m)

    nc.compile()
    return nc


def _host_shards(x, w_in, w_out, k_cache, v_cache):
    """Per-core input dicts, pre-permuted for contiguous [128, N] DMAs."""
    x2 = np.ascontiguousarray(x.reshape(BS, E))
    xT_perm = (
        x2.T.reshape(KCH, 128, 128).transpose(1, 0, 2).reshape(128, KCH * 128)
    ).astype(F16)

    # rope tables (identical on all cores); q blocks carry the attention
    # scale, the k block carries the w_kv fp8 unscale
    inv_freq = 1.0 / (ROPE_BASE ** (np.arange(0, HD, 2, dtype=np.float64) / HD))
    pos = (OFFSET + np.arange(S)).astype(np.float64)
    ang = pos[:, None] * inv_freq[None, :]
    cos16 = np.cos(ang).astype(np.float32)
    sin16 = np.sin(ang).astype(np.float32)
    scale = np.float32(1.0 / np.sqrt(HD))
    C = np.zeros((128, 640), np.float32)
    Sn = np.zeros((128, 640), np.float32)
    srow = np.arange(128) % S                        # partition p=(b,s) -> s
    for blk in range(5):
        blk_scale = scale if blk < 4 else np.float32(1.0)
        C[:, blk * 128 : blk * 128 + 64] = cos16[srow] * blk_scale
        C[:, blk * 128 + 64 : blk * 128 + 128] = cos16[srow] * blk_scale
        Sn[:, blk * 128 : blk * 128 + 64] = -sin16[srow] * blk_scale
        Sn[:, blk * 128 + 64 : blk * 128 + 128] = sin16[srow] * blk_scale

    shards = []
    for g in range(NCORES):
        wq = w_in[QF * g : QF * (g + 1)]             # [512, 4096]
        wqT = (
            wq.T.reshape(KCH, 128, 512).transpose(1, 0, 2).reshape(128, KCH * 512)
        ).astype(F16)
        wkv = np.concatenate(
            [
                w_in[E + HD * g : E + HD * (g + 1)],
                w_in[E + HKV * HD + HD * g : E + HKV * HD + HD * (g + 1)],
            ],
            axis=0,
        )                                            # [256, 4096]
        wkvT = (
            wkv.T.reshape(KCH, 128, 256).transpose(1, 0, 2).reshape(128, KCH * 256)
        ).astype(F16)
        # [128(d), (r, e)] -> [128(d), (n, r, 512)] so out-proj slice n is
        # contiguous
        w_outT_perm = (
            w_out[:, QF * g : QF * (g + 1)]
            .T.reshape(4, 128, E)
            .transpose(1, 0, 2)
            .reshape(128, 4, 8, 512)
            .transpose(0, 2, 1, 3)
            .reshape(128, 4 * E)
        ).astype(F16)
        kT = np.ascontiguousarray(
            k_cache[:, :, g, :].transpose(0, 2, 1)
        )  # [B, 128(d), T]
        k8 = kT[:, :, :K8T].astype(F8)
        k16 = kT[:, :, K8T:].astype(F16)
        vperm = np.ascontiguousarray(
            v_cache[:, :, g, :]
            .reshape(B, TCH, 128, HD)
            .transpose(0, 2, 1, 3)
            .reshape(B, 128, T)
        ).astype(F16)  # [B, 128(t_in), (chunk d)]
        shards.append(
            {
                "xT": xT_perm,
                "w_qT": wqT,
                "w_kvT": wkvT,
                "w_outT": w_outT_perm,
                "rope_c": C,
                "rope_s": Sn,
                "k8": np.ascontiguousarray(k8),
                "k16": np.ascontiguousarray(k16),
                "vperm": vperm,
            }
        )
    return shards


def _get_nc():
    if "nc" not in _CACHED:
        _CACHED["nc"] = _build_program()
    return _CACHED["nc"]


def run_on_hw(in_maps, trace=False, **kw):
    from concourse import bass_utils

    nc = _get_nc()
    return bass_utils.run_bass_kernel_spmd(
        nc, in_maps, core_ids=list(range(NCORES)), trace=trace, **kw
    )


def kernel(x, w_in, w_out, k_cache, v_cache, offset):
    assert int(offset) == OFFSET and x.shape == (B, S, E)
    shards = _host_shards(
        np.asarray(x, np.float32),
        np.asarray(w_in, np.float32),
        np.asarray(w_out, np.float32),
        np.asarray(k_cache, np.float32),
        np.asarray(v_cache, np.float32),
    )
    res = run_on_hw(shards)
    out = np.zeros((BS, E), np.float64)
    for g in range(NCORES):
        out += np.asarray(res.results[g]["out"], np.float64)
    return out.astype(np.float32).reshape(B, S, E)
